# revision 7
# baseline (speedup 1.0000x reference)
"""Multi-head attention (B=4, L=2048, E=1024, H=16, DK=64) on 8 TRN2 cores.

Sharding: core c -> (batch b = c//2, head-group g = c%2 of 8 heads).
Per core: qkv projection for its batch+heads (bf16 matmuls, fp32 accum),
attention for 8 (b,h) pairs, partial fc over its 512 features, then a
pair-wise ReduceScatter so each core emits a disjoint 1024-token slice
of the final output. Host assembles the full [4, 2048, 1024] output.

Self-contained: hardcodes all shapes; requires only the concourse stack.
"""

import numpy as np
import ml_dtypes

try:
    import axon_prof

    axon_prof.install()
except Exception:
    pass

import concourse.mybir as mybir
import concourse.tile as tile
from concourse import bacc
from concourse import bass_utils

B, L, E = 4, 2048, 1024
H, DK = 16, 64
H8 = 8                      # heads per core
F = H8 * 3 * DK             # qkv features per core = 1536
FO = H8 * DK                # attn-out features per core = 512
NCORES = 8
LHALF = L // 2

f32 = mybir.dt.float32
bf16 = mybir.dt.bfloat16
Exp = mybir.ActivationFunctionType.Exp
MUL = mybir.AluOpType.mult
ADD = mybir.AluOpType.add

_CACHE = {}


def build_nc():
    nc = bacc.Bacc("TRN2", target_bir_lowering=False, debug=False, num_devices=NCORES)

    x = nc.dram_tensor("x", [L, E], bf16, kind="ExternalInput")
    w_qkv = nc.dram_tensor("w_qkv", [E, F], bf16, kind="ExternalInput")
    b_qkv = nc.dram_tensor("b_qkv", [128, 12], f32, kind="ExternalInput")
    w_fc = nc.dram_tensor("w_fc", [FO, E], bf16, kind="ExternalInput")
    b_fc = nc.dram_tensor("b_fc", [1, E], f32, kind="ExternalInput")
    out = nc.dram_tensor("out", [LHALF, E], f32, kind="ExternalOutput")

    with tile.TileContext(nc) as tc:
        with (
            tc.tile_pool(name="persist", bufs=1) as pp,
            tc.tile_pool(name="work", bufs=2) as wp,
            tc.tile_pool(name="stage", bufs=1) as sp,
            tc.tile_pool(name="ys", bufs=3) as yp_pool,
            tc.tile_pool(name="dram", bufs=1, space="DRAM") as dram,
        ):
            # ---- persistent SBUF ----
            xT = pp.tile([128, 8, L], bf16, tag="xT")          # X^T  4 MiB
            wq = pp.tile([128, 8, F], bf16, tag="wq")          # 3 MiB
            bq = pp.tile([128, 12], f32, tag="bq")
            wfc = pp.tile([128, 4, E], bf16, tag="wfc")        # 1 MiB
            bias = pp.tile([128, E], f32, tag="bias")          # 0.5 MiB
            ones = pp.tile([128, 1], bf16, tag="ones")
            qt = pp.tile([128, 4, L], bf16, tag="qt")          # Q^T 2 MiB
            kt = pp.tile([128, 4, L], bf16, tag="kt")          # K^T 2 MiB
            # V^T staging shares its slot with the per-(j,qb) norm scratch
            vt = sp.tile([128, 4, L], bf16, tag="stage")       # V^T staging 2 MiB
            v = pp.tile([128, H8, 16, DK], bf16, tag="v")      # V natural 2 MiB
            onT = pp.tile([128, 4, L], bf16, tag="onT")        # attn out^T 2 MiB

            # ---- input DMAs ----
            for e in range(8):
                nc.sync.dma_start_transpose(xT[:, e, :], x[:, e * 128 : (e + 1) * 128])
            nc.sync.dma_start(wq[:], w_qkv.rearrange("(kc p) f -> p kc f", p=128))
            nc.sync.dma_start(bq[:], b_qkv[:])
            nc.sync.dma_start(wfc[:], w_fc.rearrange("(c p) e -> p c e", p=128))
            bfc_row = pp.tile([1, E], f32, tag="bfc_row")
            nc.sync.dma_start(bfc_row[:], b_fc[:])
            nc.gpsimd.partition_broadcast(bias[:], bfc_row[:])
            nc.vector.memset(ones[:], 1.0)

            # ---- phase 1: qkv^T = W_shard.T @ X^T (+bias), bf16 ----
            with tc.tile_pool(name="psq", bufs=2, space="PSUM") as psq_pool:
                for ft in range(12):
                    ps = psq_pool.tile([128, L], f32, tag="psq")
                    for kc in range(8):
                        lhsT = wq[:, kc, ft * 128 : (ft + 1) * 128]
                        for tb in range(4):
                            nc.tensor.matmul(
                                ps[:, tb * 512 : (tb + 1) * 512],
                                lhsT,
                                xT[:, kc, tb * 512 : (tb + 1) * 512],
                                start=(kc == 0),
                                stop=(kc == 7),
                            )
                    if ft < 4:
                        dst = qt[:, ft, :]
                    elif ft < 8:
                        dst = kt[:, ft - 4, :]
                    else:
                        dst = vt[:, ft - 8, :]
                    nc.vector.tensor_scalar_add(dst, ps[:], bq[:, ft : ft + 1])

            # V^T -> V (token-major) via xbar transpose, per head
            for h in range(H8):
                nc.sync.dma_start_transpose(
                    v[:, h, :, :],
                    vt[(h % 2) * 64 : (h % 2) * 64 + 64, h // 2, :],
                )

            # ---- phase 2: attention, head-pair j = (2j, 2j+1), q in halves ----
            with (
                tc.tile_pool(name="pst", bufs=1, space="PSUM") as pst_pool,
                tc.tile_pool(name="psav", bufs=1, space="PSUM") as psav_pool,
                tc.tile_pool(name="pssm", bufs=1, space="PSUM") as pssm_pool,
            ):
                for j in range(4):
                    for qb in range(2):
                        av = psav_pool.tile([128, LHALF], f32, tag="av")
                        sm = pssm_pool.tile([128, LHALF], f32, tag="sm")
                        for kk in range(16):
                            st = pst_pool.tile([128, L], f32, tag="st")
                            # S^T for both heads; row groups 0-63 / 64-127
                            # run concurrently on the PE
                            for u in range(2):
                                q0 = qb * LHALF + u * 512
                                nc.tensor.matmul(
                                    st[:, u * 512 : (u + 1) * 512],
                                    kt[0:64, j, kk * 128 : (kk + 1) * 128],
                                    qt[0:64, j, q0 : q0 + 512],
                                    start=True,
                                    stop=True,
                                )
                                nc.tensor.matmul(
                                    st[:, 1024 + u * 512 : 1024 + (u + 1) * 512],
                                    kt[64:128, j, kk * 128 : (kk + 1) * 128],
                                    qt[64:128, j, q0 : q0 + 512],
                                    start=True,
                                    stop=True,
                                )
                            pt = wp.tile([128, L], bf16, tag="pt")
                            nc.scalar.activation(
                                pt[:, 0:1024], st[:, 0:1024], Exp, scale=0.125
                            )
                            nc.scalar.activation(
                                pt[:, 1024:2048], st[:, 1024:2048], Exp, scale=0.125
                            )
                            # AV + rowsums; sums ride free column strips
                            first, last = kk == 0, kk == 15
                            for u in range(2):
                                sl = slice(u * 512, (u + 1) * 512)
                                sr = slice(1024 + u * 512, 1024 + (u + 1) * 512)
                                nc.tensor.matmul(
                                    av[0:64, sl], v[:, 2 * j, kk, :], pt[:, sl],
                                    start=first, stop=last,
                                )
                                nc.tensor.matmul(
                                    sm[64:65, sl], ones[:], pt[:, sr],
                                    start=first, stop=last,
                                )
                                nc.tensor.matmul(
                                    av[64:128, sl], v[:, 2 * j + 1, kk, :], pt[:, sr],
                                    start=first, stop=last,
                                )
                                nc.tensor.matmul(
                                    sm[0:1, sl], ones[:], pt[:, sl],
                                    start=first, stop=last,
                                )
                        # normalize: out^T rows /= rowsum.
                        # srs cols 0:2048 = staging row, 2048:4096 = broadcast
                        srs = sp.tile([128, 2 * L], f32, tag="stage")
                        nc.vector.tensor_copy(srs[0:1, 0:1024], sm[0:1, :])
                        nc.vector.tensor_copy(srs[64:65, 1024:2048], sm[64:65, :])
                        nc.sync.dma_start(
                            srs[0:1, 1024:2048], srs[64:65, 1024:2048]
                        )
                        nc.gpsimd.partition_broadcast(
                            srs[:, 2048:4096], srs[0:1, 0:2048]
                        )
                        nc.vector.reciprocal(srs[:, 2048:4096], srs[:, 2048:4096])
                        qsl = slice(qb * LHALF, (qb + 1) * LHALF)
                        nc.vector.tensor_tensor(
                            onT[0:64, j, qsl], av[0:64, :],
                            srs[0:64, 2048:3072], op=MUL,
                        )
                        nc.vector.tensor_tensor(
                            onT[64:128, j, qsl], av[64:128, :],
                            srs[64:128, 3072:4096], op=MUL,
                        )

            # ---- phase 3: partial fc + pair ReduceScatter ----
            rs_in = dram.tile([L, E], f32)
            rs_out = dram.tile([LHALF, E], f32)
            with tc.tile_pool(name="psy", bufs=2, space="PSUM") as psy_pool:
                for t in range(16):
                    yp = psy_pool.tile([128, E], f32, tag="yp")
                    for c in range(4):
                        lhsT = onT[:, c, t * 128 : (t + 1) * 128]
                        for e2 in range(2):
                            nc.tensor.matmul(
                                yp[:, e2 * 512 : (e2 + 1) * 512],
                                lhsT,
                                wfc[:, c, e2 * 512 : (e2 + 1) * 512],
                                start=(c == 0),
                                stop=(c == 3),
                            )
                    ys = yp_pool.tile([128, E], f32, tag="ys")
                    nc.vector.tensor_tensor(ys[:], yp[:], bias[:], op=ADD)
                    nc.sync.dma_start(rs_in[t * 128 : (t + 1) * 128, :], ys[:])
            nc.gpsimd.collective_compute(
                "ReduceScatter",
                ADD,
                replica_groups=[[0, 1], [2, 3], [4, 5], [6, 7]],
                ins=[rs_in.opt()],
                outs=[rs_out.opt()],
            )
            nc.sync.dma_start(out[:], rs_out[:])

    nc.finalize()
    return nc


def _prep_inputs(X, W_qkv, b_qkv, W_fc, b_fc):
    """Host-side shard + permute + cast. Returns in_maps for 8 cores."""
    X = np.asarray(X, dtype=np.float32)
    W_qkv = np.asarray(W_qkv, dtype=np.float32)
    b_qkv = np.asarray(b_qkv, dtype=np.float32)
    W_fc = np.asarray(W_fc, dtype=np.float32)
    b_fc = np.asarray(b_fc, dtype=np.float32)

    in_maps = []
    bfc_half = (0.5 * b_fc).astype(np.float32).reshape(1, E)
    for c in range(NCORES):
        b, g = divmod(c, 2)
        heads = np.arange(g * H8, (g + 1) * H8)
        # column order: all Q feats (head-major), then K, then V
        cols = np.concatenate(
            [
                np.concatenate([h * 3 * DK + off + np.arange(DK) for h in heads])
                for off in (0, DK, 2 * DK)
            ]
        )
        wq_sh = W_qkv[:, cols].astype(ml_dtypes.bfloat16)
        bq_sh = b_qkv[cols].astype(np.float32).reshape(12, 128).T.copy()
        wfc_sh = W_fc[g * FO : (g + 1) * FO, :].astype(ml_dtypes.bfloat16)
        in_maps.append(
            {
                "x": X[b].astype(ml_dtypes.bfloat16),
                "w_qkv": wq_sh,
                "b_qkv": np.ascontiguousarray(bq_sh),
                "w_fc": wfc_sh,
                "b_fc": bfc_half,
            }
        )
    return in_maps


def run_kernel(inputs, trace=False):
    if "nc" not in _CACHE:
        _CACHE["nc"] = build_nc()
    nc = _CACHE["nc"]
    in_maps = _prep_inputs(**inputs)
    res = bass_utils.run_bass_kernel_spmd(
        nc, in_maps, core_ids=list(range(NCORES)), trace=trace
    )
    Y = np.empty((B, L, E), dtype=np.float32)
    for c in range(NCORES):
        b, g = divmod(c, 2)
        Y[b, g * LHALF : (g + 1) * LHALF, :] = res.results[c]["out"]
    return Y, res


def kernel(X, W_qkv, b_qkv, W_fc, b_fc):
    Y, _ = run_kernel(
        dict(X=X, W_qkv=W_qkv, b_qkv=b_qkv, W_fc=W_fc, b_fc=b_fc), trace=False
    )
    return Y


# revision 8
# speedup vs baseline: 1.9016x; 1.9016x over previous
"""Multi-head attention (B=4, L=2048, E=1024, H=16, DK=64) on 8 TRN2 cores.

Sharding: core c -> (batch b = c//2, head-group g = c%2 of 8 heads).
Per core: qkv projection for its batch+heads (bf16 matmuls, fp32 accum),
attention for 8 (b,h) pairs, partial fc over its 512 features, then a
pair-wise ReduceScatter so each core emits a disjoint 1024-token slice
of the final output. Host assembles the full [4, 2048, 1024] output.

Self-contained: hardcodes all shapes; requires only the concourse stack.
"""

import numpy as np
import ml_dtypes

try:
    import axon_prof

    axon_prof.install()
except Exception:
    pass

import concourse.mybir as mybir
import concourse.tile as tile
from concourse import bacc
from concourse import bass_utils

B, L, E = 4, 2048, 1024
H, DK = 16, 64
H8 = 8                      # heads per core
F = H8 * 3 * DK             # qkv features per core = 1536
FO = H8 * DK                # attn-out features per core = 512
NCORES = 8
LHALF = L // 2

f32 = mybir.dt.float32
bf16 = mybir.dt.bfloat16
Exp = mybir.ActivationFunctionType.Exp
MUL = mybir.AluOpType.mult
ADD = mybir.AluOpType.add

_CACHE = {}


def build_nc():
    nc = bacc.Bacc("TRN2", target_bir_lowering=False, debug=False, num_devices=NCORES)

    x = nc.dram_tensor("x", [L, E], bf16, kind="ExternalInput")
    w_qkv = nc.dram_tensor("w_qkv", [E, F], bf16, kind="ExternalInput")
    b_qkv = nc.dram_tensor("b_qkv", [128, 12], f32, kind="ExternalInput")
    w_fc = nc.dram_tensor("w_fc", [FO, E], bf16, kind="ExternalInput")
    b_fc = nc.dram_tensor("b_fc", [1, E], f32, kind="ExternalInput")
    out = nc.dram_tensor("out", [LHALF, E], f32, kind="ExternalOutput")

    with tile.TileContext(nc) as tc:
        with (
            tc.tile_pool(name="persist", bufs=1) as pp,
            tc.tile_pool(name="work", bufs=2) as wp,
            tc.tile_pool(name="stage", bufs=1) as sp,
            tc.tile_pool(name="ys", bufs=3) as yp_pool,
            tc.tile_pool(name="dram", bufs=1, space="DRAM") as dram,
        ):
            # ---- persistent SBUF ----
            xT = pp.tile([128, 8, L], bf16, tag="xT")          # X^T  4 MiB
            wq = pp.tile([128, 8, F], bf16, tag="wq")          # 3 MiB
            bq = pp.tile([128, 12], f32, tag="bq")
            wfc = pp.tile([128, 4, E], bf16, tag="wfc")        # 1 MiB
            bias = pp.tile([128, E], f32, tag="bias")          # 0.5 MiB
            ones = pp.tile([128, 1], bf16, tag="ones")
            qt = pp.tile([128, 4, L], bf16, tag="qt")          # Q^T 2 MiB
            kt = pp.tile([128, 4, L], bf16, tag="kt")          # K^T 2 MiB
            # V^T staging shares its slot with the per-(j,qb) norm scratch
            vt = sp.tile([128, 4, L], bf16, tag="stage")       # V^T staging 2 MiB
            v = pp.tile([128, H8, 16, DK], bf16, tag="v")      # V natural 2 MiB
            onT = pp.tile([128, 4, L], bf16, tag="onT")        # attn out^T 2 MiB

            # ---- input DMAs ----
            for e in range(8):
                nc.sync.dma_start_transpose(xT[:, e, :], x[:, e * 128 : (e + 1) * 128])
            nc.sync.dma_start(wq[:], w_qkv.rearrange("(kc p) f -> p kc f", p=128))
            nc.sync.dma_start(bq[:], b_qkv[:])
            nc.sync.dma_start(wfc[:], w_fc.rearrange("(c p) e -> p c e", p=128))
            bfc_row = pp.tile([1, E], f32, tag="bfc_row")
            nc.sync.dma_start(bfc_row[:], b_fc[:])
            nc.gpsimd.partition_broadcast(bias[:], bfc_row[:])
            nc.vector.memset(ones[:], 1.0)

            # ---- phase 1: qkv^T = W_shard.T @ X^T (+bias), bf16 ----
            with tc.tile_pool(name="psq", bufs=2, space="PSUM") as psq_pool:
                for ft in range(12):
                    ps = psq_pool.tile([128, L], f32, tag="psq")
                    for kc in range(8):
                        lhsT = wq[:, kc, ft * 128 : (ft + 1) * 128]
                        for tb in range(4):
                            nc.tensor.matmul(
                                ps[:, tb * 512 : (tb + 1) * 512],
                                lhsT,
                                xT[:, kc, tb * 512 : (tb + 1) * 512],
                                start=(kc == 0),
                                stop=(kc == 7),
                            )
                    if ft < 4:
                        dst = qt[:, ft, :]
                    elif ft < 8:
                        dst = kt[:, ft - 4, :]
                    else:
                        dst = vt[:, ft - 8, :]
                    nc.vector.tensor_scalar_add(dst, ps[:], bq[:, ft : ft + 1])

            # V^T -> V (token-major) via xbar transpose, per head
            for h in range(H8):
                nc.sync.dma_start_transpose(
                    v[:, h, :, :],
                    vt[(h % 2) * 64 : (h % 2) * 64 + 64, h // 2, :],
                )

            # ---- phase 2: attention, head-pair j = (2j, 2j+1), q in halves ----
            # Software-pipelined: AV(kk-1) is emitted after S^T(kk)/exp(kk)
            # so the PE computes AV while ACT runs exp of the next chunk.
            with (
                tc.tile_pool(name="pst", bufs=1, space="PSUM") as pst_pool,
                tc.tile_pool(name="psav", bufs=1, space="PSUM") as psav_pool,
                tc.tile_pool(name="pssm", bufs=1, space="PSUM") as pssm_pool,
            ):
                for qb in range(2):
                    for j in range(4):
                        av = psav_pool.tile([128, LHALF], f32, tag="av")
                        sm = pssm_pool.tile([128, LHALF], f32, tag="sm")
                        pts = {}

                        def emit_st(kk):
                            st = pst_pool.tile([128, L], f32, tag="st")
                            for u in range(2):
                                q0 = qb * LHALF + u * 512
                                nc.tensor.matmul(
                                    st[:, u * 512 : (u + 1) * 512],
                                    kt[0:64, j, kk * 128 : (kk + 1) * 128],
                                    qt[0:64, j, q0 : q0 + 512],
                                    start=True,
                                    stop=True,
                                )
                                nc.tensor.matmul(
                                    st[:, 1024 + u * 512 : 1024 + (u + 1) * 512],
                                    kt[64:128, j, kk * 128 : (kk + 1) * 128],
                                    qt[64:128, j, q0 : q0 + 512],
                                    start=True,
                                    stop=True,
                                )
                            pt = wp.tile([128, L], bf16, tag="pt")
                            nc.scalar.activation(
                                pt[:, 0:1024], st[:, 0:1024], Exp, scale=0.125
                            )
                            nc.scalar.activation(
                                pt[:, 1024:2048], st[:, 1024:2048], Exp, scale=0.125
                            )
                            pts[kk] = pt

                        def emit_av(kk):
                            pt = pts.pop(kk)
                            first, last = kk == 0, kk == 15
                            for u in range(2):
                                sl = slice(u * 512, (u + 1) * 512)
                                sr = slice(1024 + u * 512, 1024 + (u + 1) * 512)
                                nc.tensor.matmul(
                                    av[0:64, sl], v[:, 2 * j, kk, :], pt[:, sl],
                                    start=first, stop=last,
                                )
                                nc.tensor.matmul(
                                    sm[64:65, sl], ones[:], pt[:, sr],
                                    start=first, stop=last,
                                )
                                nc.tensor.matmul(
                                    av[64:128, sl], v[:, 2 * j + 1, kk, :], pt[:, sr],
                                    start=first, stop=last,
                                )
                                nc.tensor.matmul(
                                    sm[0:1, sl], ones[:], pt[:, sl],
                                    start=first, stop=last,
                                )

                        for kk in range(16):
                            emit_st(kk)
                            if kk > 0:
                                emit_av(kk - 1)
                        emit_av(15)

                        # early evict (frees av/sm psum): unnormalized out^T
                        qsl = slice(qb * LHALF, (qb + 1) * LHALF)
                        nc.vector.tensor_copy(onT[0:64, j, qsl], av[0:64, :])
                        nc.vector.tensor_copy(onT[64:128, j, qsl], av[64:128, :])
                        srs = sp.tile([128, 2 * L], f32, tag="stage")
                        nc.vector.tensor_copy(srs[0:1, 0:1024], sm[0:1, :])
                        nc.vector.tensor_copy(srs[64:65, 1024:2048], sm[64:65, :])
                        # deferred normalization (overlaps the next block):
                        # srs cols 0:2048 = sums row, 2048:4096 = broadcast
                        nc.sync.dma_start(
                            srs[0:1, 1024:2048], srs[64:65, 1024:2048]
                        )
                        nc.gpsimd.partition_broadcast(
                            srs[:, 2048:4096], srs[0:1, 0:2048]
                        )
                        nc.vector.reciprocal_approx_fast(
                            srs[:, 2048:4096], srs[:, 2048:4096]
                        )
                        nc.vector.tensor_tensor(
                            onT[0:64, j, qsl], onT[0:64, j, qsl],
                            srs[0:64, 2048:3072], op=MUL,
                        )
                        nc.vector.tensor_tensor(
                            onT[64:128, j, qsl], onT[64:128, j, qsl],
                            srs[64:128, 3072:4096], op=MUL,
                        )

            # ---- phase 3: partial fc (bf16 out) + pair ReduceScatter ----
            rs_in = dram.tile([L, E], bf16)
            rs_out = dram.tile([LHALF, E], bf16)
            with tc.tile_pool(name="psy", bufs=2, space="PSUM") as psy_pool:
                for t in range(16):
                    yp = psy_pool.tile([128, E], f32, tag="yp")
                    for c in range(4):
                        lhsT = onT[:, c, t * 128 : (t + 1) * 128]
                        for e2 in range(2):
                            nc.tensor.matmul(
                                yp[:, e2 * 512 : (e2 + 1) * 512],
                                lhsT,
                                wfc[:, c, e2 * 512 : (e2 + 1) * 512],
                                start=(c == 0),
                                stop=(c == 3),
                            )
                    ys = yp_pool.tile([128, E], bf16, tag="ys")
                    nc.vector.tensor_tensor(ys[:], yp[:], bias[:], op=ADD)
                    nc.sync.dma_start(rs_in[t * 128 : (t + 1) * 128, :], ys[:])
            nc.gpsimd.collective_compute(
                "ReduceScatter",
                ADD,
                replica_groups=[[0, 1], [2, 3], [4, 5], [6, 7]],
                ins=[rs_in.opt()],
                outs=[rs_out.opt()],
            )
            # SWDGE cast bf16 -> f32 on the way out
            nc.gpsimd.dma_start(out[:], rs_out[:])

    nc.finalize()
    return nc


def _prep_inputs(X, W_qkv, b_qkv, W_fc, b_fc):
    """Host-side shard + permute + cast. Returns in_maps for 8 cores."""
    X = np.asarray(X, dtype=np.float32)
    W_qkv = np.asarray(W_qkv, dtype=np.float32)
    b_qkv = np.asarray(b_qkv, dtype=np.float32)
    W_fc = np.asarray(W_fc, dtype=np.float32)
    b_fc = np.asarray(b_fc, dtype=np.float32)

    in_maps = []
    bfc_half = (0.5 * b_fc).astype(np.float32).reshape(1, E)
    for c in range(NCORES):
        b, g = divmod(c, 2)
        heads = np.arange(g * H8, (g + 1) * H8)
        # column order: all Q feats (head-major), then K, then V
        cols = np.concatenate(
            [
                np.concatenate([h * 3 * DK + off + np.arange(DK) for h in heads])
                for off in (0, DK, 2 * DK)
            ]
        )
        wq_sh = W_qkv[:, cols].astype(ml_dtypes.bfloat16)
        bq_sh = b_qkv[cols].astype(np.float32).reshape(12, 128).T.copy()
        wfc_sh = W_fc[g * FO : (g + 1) * FO, :].astype(ml_dtypes.bfloat16)
        in_maps.append(
            {
                "x": X[b].astype(ml_dtypes.bfloat16),
                "w_qkv": wq_sh,
                "b_qkv": np.ascontiguousarray(bq_sh),
                "w_fc": wfc_sh,
                "b_fc": bfc_half,
            }
        )
    return in_maps


def run_kernel(inputs, trace=False):
    if "nc" not in _CACHE:
        _CACHE["nc"] = build_nc()
    nc = _CACHE["nc"]
    in_maps = _prep_inputs(**inputs)
    res = bass_utils.run_bass_kernel_spmd(
        nc, in_maps, core_ids=list(range(NCORES)), trace=trace
    )
    Y = np.empty((B, L, E), dtype=np.float32)
    for c in range(NCORES):
        b, g = divmod(c, 2)
        Y[b, g * LHALF : (g + 1) * LHALF, :] = res.results[c]["out"]
    return Y, res


def kernel(X, W_qkv, b_qkv, W_fc, b_fc):
    Y, _ = run_kernel(
        dict(X=X, W_qkv=W_qkv, b_qkv=b_qkv, W_fc=W_fc, b_fc=b_fc), trace=False
    )
    return Y


# revision 16
# speedup vs baseline: 2.0207x; 1.0626x over previous
"""Multi-head attention (B=4, L=2048, E=1024, H=16, DK=64) on 8 TRN2 cores.

Sharding: core c -> (batch b = c//2, head-group g = c%2 of 8 heads).
Per core: qkv projection for its batch+heads (bf16 matmuls, fp32 accum),
attention for 8 (b,h) pairs, partial fc over its 512 features, then a
pair-wise ReduceScatter so each core emits a disjoint 1024-token slice
of the final output. Host assembles the full [4, 2048, 1024] output.

Self-contained: hardcodes all shapes; requires only the concourse stack.
"""

import numpy as np
import ml_dtypes

try:
    import axon_prof

    axon_prof.install()
except Exception:
    pass

import concourse.mybir as mybir
import concourse.tile as tile
from concourse import bacc
from concourse import bass_utils

B, L, E = 4, 2048, 1024
H, DK = 16, 64
H8 = 8                      # heads per core
F = H8 * 3 * DK             # qkv features per core = 1536
FO = H8 * DK                # attn-out features per core = 512
NCORES = 8
LHALF = L // 2

f32 = mybir.dt.float32
bf16 = mybir.dt.bfloat16
Exp = mybir.ActivationFunctionType.Exp
MUL = mybir.AluOpType.mult
ADD = mybir.AluOpType.add

_CACHE = {}


def build_nc():
    nc = bacc.Bacc("TRN2", target_bir_lowering=False, debug=False, num_devices=NCORES)

    x = nc.dram_tensor("x", [L, E], bf16, kind="ExternalInput")
    w_qkv = nc.dram_tensor("w_qkv", [E, F], bf16, kind="ExternalInput")
    b_qkv = nc.dram_tensor("b_qkv", [128, 12], f32, kind="ExternalInput")
    w_fc = nc.dram_tensor("w_fc", [FO, E], bf16, kind="ExternalInput")
    b_fc = nc.dram_tensor("b_fc", [1, E], f32, kind="ExternalInput")
    out = nc.dram_tensor("out", [LHALF, E], f32, kind="ExternalOutput")

    with tile.TileContext(nc) as tc:
        with (
            tc.tile_pool(name="persist", bufs=1) as pp,
            tc.tile_pool(name="work", bufs=2) as wp,
            tc.tile_pool(name="stage", bufs=1) as sp,
            tc.tile_pool(name="ys", bufs=3) as yp_pool,
            tc.tile_pool(name="dram", bufs=1, space="DRAM") as dram,
        ):
            # ---- persistent SBUF ----
            xT = pp.tile([128, 8, L], bf16, tag="xT")          # X^T  4 MiB
            wq = pp.tile([128, 8, F], bf16, tag="wq")          # 3 MiB
            bq = pp.tile([128, 12], f32, tag="bq")
            wfc = pp.tile([128, 4, E], bf16, tag="wfc")        # 1 MiB
            bias = pp.tile([128, E], f32, tag="bias")          # 0.5 MiB
            qt = pp.tile([128, 4, L], bf16, tag="qt")          # Q^T 2 MiB
            kt = pp.tile([128, 4, L], bf16, tag="kt")          # K^T 2 MiB
            vt = sp.tile([128, 4, L], bf16, tag="vt")          # V^T staging 2 MiB
            # V natural layout, 80-elem stride; col 64 holds the ones column
            # so AV matmuls with lhsT [V|1] (M=65) produce rowsums for free
            v = pp.tile([128, H8, 16, 80], bf16, tag="v")      # 2.5 MiB
            onT = pp.tile([128, 4, L], bf16, tag="onT")        # attn out^T 2 MiB

            # ---- input DMAs (interleaved so qkv matmuls start early) ----
            for e in range(8):
                nc.sync.dma_start(
                    wq[:, e, :], w_qkv[e * 128 : (e + 1) * 128, :]
                )
                nc.sync.dma_start_transpose(xT[:, e, :], x[:, e * 128 : (e + 1) * 128])
                if e == 0:
                    nc.sync.dma_start(bq[:], b_qkv[:])
            nc.sync.dma_start(wfc[:], w_fc.rearrange("(c p) e -> p c e", p=128))
            bfc_row = pp.tile([1, E], f32, tag="bfc_row")
            nc.sync.dma_start(bfc_row[:], b_fc[:])
            nc.gpsimd.partition_broadcast(bias[:], bfc_row[:])
            nc.vector.memset(v[:, :, :, 64:65], 1.0)

            # ---- phase 1: qkv^T = W_shard.T @ X^T (+bias), bf16 ----
            with tc.tile_pool(name="psq", bufs=2, space="PSUM") as psq_pool:
                for ft in range(12):
                    ps = psq_pool.tile([128, L], f32, tag="psq")
                    for kc in range(8):
                        lhsT = wq[:, kc, ft * 128 : (ft + 1) * 128]
                        for tb in range(4):
                            nc.tensor.matmul(
                                ps[:, tb * 512 : (tb + 1) * 512],
                                lhsT,
                                xT[:, kc, tb * 512 : (tb + 1) * 512],
                                start=(kc == 0),
                                stop=(kc == 7),
                            )
                    if ft < 4:
                        dst = qt[:, ft, :]
                    elif ft < 8:
                        dst = kt[:, ft - 4, :]
                    else:
                        dst = vt[:, ft - 8, :]
                    nc.vector.tensor_scalar_add(dst, ps[:], bq[:, ft : ft + 1])

            # V^T -> V (token-major) via xbar transpose, per head
            for h in range(H8):
                nc.sync.dma_start_transpose(
                    v[:, h, :, 0:DK],
                    vt[(h % 2) * 64 : (h % 2) * 64 + 64, h // 2, :],
                )

            # ---- phase 2+3: attention halves, each followed by partial fc
            # and a pair ReduceScatter (RS#1 overlaps the qb=1 half) ----
            # Attention is software-pipelined: AV(kk-1) is emitted after
            # S^T(kk)/exp(kk) so the PE computes AV while ACT runs exp.
            rs_in = [
                dram.tile([LHALF, E], bf16, name=f"rs_in{i}", tag=f"rs_in{i}")
                for i in range(2)
            ]
            rs_out = [
                dram.tile([LHALF // 2, E], bf16, name=f"rs_out{i}", tag=f"rs_out{i}")
                for i in range(2)
            ]
            with (
                tc.tile_pool(name="pst", bufs=1, space="PSUM") as pst_pool,
                tc.tile_pool(name="psav0", bufs=1, space="PSUM") as psav0_pool,
                tc.tile_pool(name="psav1", bufs=1, space="PSUM") as psav1_pool,
            ):

                def attn_block(qb, j):
                    av0 = psav0_pool.tile([128, LHALF], f32, tag="av0")
                    av1 = psav1_pool.tile([128, LHALF], f32, tag="av1")
                    pts = {}

                    def emit_st(kk):
                        st = pst_pool.tile([128, L], f32, tag="st")
                        for u in range(2):
                            q0 = qb * LHALF + u * 512
                            nc.tensor.matmul(
                                st[:, u * 512 : (u + 1) * 512],
                                kt[0:64, j, kk * 128 : (kk + 1) * 128],
                                qt[0:64, j, q0 : q0 + 512],
                                start=True,
                                stop=True,
                            )
                            nc.tensor.matmul(
                                st[:, 1024 + u * 512 : 1024 + (u + 1) * 512],
                                kt[64:128, j, kk * 128 : (kk + 1) * 128],
                                qt[64:128, j, q0 : q0 + 512],
                                start=True,
                                stop=True,
                            )
                        pt = wp.tile([128, L], bf16, tag="pt")
                        nc.scalar.activation(
                            pt[:, 0:1024], st[:, 0:1024], Exp, scale=0.125
                        )
                        nc.scalar.activation(
                            pt[:, 1024:2048], st[:, 1024:2048], Exp, scale=0.125
                        )
                        pts[kk] = pt

                    def emit_av(kk):
                        pt = pts.pop(kk)
                        first, last = kk == 0, kk == 15
                        for u in range(2):
                            sl = slice(u * 512, (u + 1) * 512)
                            sr = slice(1024 + u * 512, 1024 + (u + 1) * 512)
                            nc.tensor.matmul(
                                av0[0:65, sl], v[:, 2 * j, kk, 0:65], pt[:, sl],
                                start=first, stop=last,
                            )
                            nc.tensor.matmul(
                                av1[0:65, sl], v[:, 2 * j + 1, kk, 0:65], pt[:, sr],
                                start=first, stop=last,
                            )

                    for kk in range(16):
                        emit_st(kk)
                        if kk > 0:
                            emit_av(kk - 1)
                    emit_av(15)

                    # early evict (frees av psum): unnormalized out^T.
                    # av1 rows 0:64 must land on partitions 64:128 -> DMA shift.
                    qsl = slice(qb * LHALF, (qb + 1) * LHALF)
                    nc.vector.tensor_copy(onT[0:64, j, qsl], av0[0:64, :])
                    tmp = wp.tile([64, LHALF], bf16, tag="tmp")
                    nc.vector.tensor_copy(tmp[:], av1[0:64, :])
                    srs = sp.tile([128, 2 * L], f32, tag="stage")
                    nc.vector.tensor_copy(srs[64:65, 0:1024], av0[64:65, :])
                    nc.vector.tensor_copy(srs[64:65, 1024:2048], av1[64:65, :])
                    # deferred normalization (overlaps the next block):
                    # srs cols 0:2048 = sums row, 2048:4096 = broadcast
                    nc.sync.dma_start(onT[64:128, j, qsl], tmp[:])
                    nc.sync.dma_start(srs[0:1, 0:2048], srs[64:65, 0:2048])
                    nc.gpsimd.partition_broadcast(
                        srs[:, 2048:4096], srs[0:1, 0:2048]
                    )
                    nc.vector.reciprocal_approx_fast(
                        srs[:, 2048:4096], srs[:, 2048:4096]
                    )
                    nc.vector.tensor_tensor(
                        onT[0:64, j, qsl], onT[0:64, j, qsl],
                        srs[0:64, 2048:3072], op=MUL,
                    )
                    nc.vector.tensor_tensor(
                        onT[64:128, j, qsl], onT[64:128, j, qsl],
                        srs[64:128, 3072:4096], op=MUL,
                    )

                def fc_half(qb):
                    # fc for this token half; psum slots borrowed from av pools
                    for t8 in range(8):
                        t = qb * 8 + t8
                        pool = psav0_pool if t8 % 2 == 0 else psav1_pool
                        tag = "av0" if t8 % 2 == 0 else "av1"
                        yp = pool.tile([128, E], f32, tag=tag)
                        for c in range(4):
                            lhsT = onT[:, c, t * 128 : (t + 1) * 128]
                            for e2 in range(2):
                                nc.tensor.matmul(
                                    yp[:, e2 * 512 : (e2 + 1) * 512],
                                    lhsT,
                                    wfc[:, c, e2 * 512 : (e2 + 1) * 512],
                                    start=(c == 0),
                                    stop=(c == 3),
                                )
                        ys = yp_pool.tile([128, E], bf16, tag="ys")
                        nc.vector.tensor_tensor(ys[:], yp[:], bias[:], op=ADD)
                        nc.sync.dma_start(
                            rs_in[qb][t8 * 128 : (t8 + 1) * 128, :], ys[:]
                        )

                for qb in range(2):
                    for j in range(4):
                        attn_block(qb, j)
                    fc_half(qb)
                    nc.gpsimd.collective_compute(
                        "ReduceScatter",
                        ADD,
                        replica_groups=[[0, 1], [2, 3], [4, 5], [6, 7]],
                        ins=[rs_in[qb].opt()],
                        outs=[rs_out[qb].opt()],
                    )

            # SWDGE cast bf16 -> f32 on the way out
            nc.gpsimd.dma_start(out[0 : LHALF // 2, :], rs_out[0][:])
            nc.gpsimd.dma_start(out[LHALF // 2 : LHALF, :], rs_out[1][:])

    nc.finalize()
    return nc


def _prep_inputs(X, W_qkv, b_qkv, W_fc, b_fc):
    """Host-side shard + permute + cast. Returns in_maps for 8 cores."""
    X = np.asarray(X, dtype=np.float32)
    W_qkv = np.asarray(W_qkv, dtype=np.float32)
    b_qkv = np.asarray(b_qkv, dtype=np.float32)
    W_fc = np.asarray(W_fc, dtype=np.float32)
    b_fc = np.asarray(b_fc, dtype=np.float32)

    in_maps = []
    bfc_half = (0.5 * b_fc).astype(np.float32).reshape(1, E)
    for c in range(NCORES):
        b, g = divmod(c, 2)
        heads = np.arange(g * H8, (g + 1) * H8)
        # column order: all Q feats (head-major), then K, then V
        cols = np.concatenate(
            [
                np.concatenate([h * 3 * DK + off + np.arange(DK) for h in heads])
                for off in (0, DK, 2 * DK)
            ]
        )
        wq_sh = W_qkv[:, cols].astype(ml_dtypes.bfloat16)
        bq_sh = b_qkv[cols].astype(np.float32).reshape(12, 128).T.copy()
        wfc_sh = W_fc[g * FO : (g + 1) * FO, :].astype(ml_dtypes.bfloat16)
        in_maps.append(
            {
                "x": X[b].astype(ml_dtypes.bfloat16),
                "w_qkv": wq_sh,
                "b_qkv": np.ascontiguousarray(bq_sh),
                "w_fc": wfc_sh,
                "b_fc": bfc_half,
            }
        )
    return in_maps


def run_kernel(inputs, trace=False):
    if "nc" not in _CACHE:
        _CACHE["nc"] = build_nc()
    nc = _CACHE["nc"]
    in_maps = _prep_inputs(**inputs)
    res = bass_utils.run_bass_kernel_spmd(
        nc, in_maps, core_ids=list(range(NCORES)), trace=trace
    )
    Y = np.empty((B, L, E), dtype=np.float32)
    Q = LHALF // 2  # 512
    for c in range(NCORES):
        b, g = divmod(c, 2)
        o = res.results[c]["out"]
        # RS#1 scattered tokens [0:1024] -> rank g got [g*Q:(g+1)*Q];
        # RS#2 scattered tokens [1024:2048] -> rank g got [1024+g*Q:...]
        Y[b, g * Q : (g + 1) * Q, :] = o[0:Q]
        Y[b, LHALF + g * Q : LHALF + (g + 1) * Q, :] = o[Q : 2 * Q]
    return Y, res


def kernel(X, W_qkv, b_qkv, W_fc, b_fc):
    Y, _ = run_kernel(
        dict(X=X, W_qkv=W_qkv, b_qkv=b_qkv, W_fc=W_fc, b_fc=b_fc), trace=False
    )
    return Y
